# revision 1
# baseline (speedup 1.0000x reference)
"""GNN message-passing (GEM-CNN style) Trainium2 kernel.

Strategy (node-sharded, scatter-first):
  - 8 cores, core c owns a contiguous slice of nodes (plus padding slots).
  - Host packs each core's nodes into 16-slot "buckets" with <=128 incident
    (incoming) edges each; per bucket a dense [128, 160] matrix A with
    A[edge_row, slot_local*10 + k] = Pf[edge, k] is prebuilt, so that a single
    PE matmul  xg_rot^T @ A  scatter-accumulates S^T[(f), (slot,k)] in PSUM.
  - Node phase: out^T[o, n] = sum_k Wk^T S_k^T (+ shortcut matmul + bias),
    then reg_relu as two more matmuls with a fused ACT relu, all in [f, n]
    layout; PE transposes convert back to row layout for the next gather.
  - AllGather of node-feature shards between convs; AllReduce of the tiny
    pooled [64, 40] logits at the end; log_softmax on-chip.

All features/weights in bf16 (validated: ~2e-4 rel err), PSUM accum fp32.
"""
import numpy as np
import ml_dtypes

import concourse.bass as bass
import concourse.bacc as bacc
import concourse.mybir as mybir
import concourse.tile as tile
from concourse.bass_utils import run_bass_kernel_spmd

BF16 = ml_dtypes.bfloat16

N_NODES, N_EDGES, G = 30000, 180000, 64
NC = 8
NPC = N_NODES // NC            # nodes per core
BUCK_SLOTS = 16                # node slots per bucket
BUCK_EDGES = 128               # max edges per bucket
K = 10                         # precomp channels (2 rings x 5 ang)
CHUNK_BUCKS = 30               # buckets per node-chunk (480 slots)
GRP = 6                        # buckets per gather/transport group
TBLK = 120                     # transpose block (slots)

# conv table: (Fin, Cin, in_order, Fout, Cout, out_order)
CONVS = [
    (35, 7, 2, 80, 16, 2),   # b1c1
    (80, 16, 2, 80, 16, 2),  # b1c2
    (80, 16, 2, 80, 16, 2),  # b2c1
    (80, 16, 2, 80, 16, 2),  # b2c2
    (80, 16, 2, 16, 16, 0),  # b3c1
    (16, 16, 0, 16, 16, 0),  # b3c2
]


# ----------------------------------------------------------------------------
# host-side helpers
# ----------------------------------------------------------------------------

def _fourier(order, S=7):
    phi = np.arange(S) * (2.0 * np.pi / S)
    D = 2 * order + 1
    B = np.ones((S, D), np.float32)
    Bi = np.full((S, D), 1.0 / S, np.float32)
    for m in range(1, order + 1):
        B[:, 2 * m - 1], B[:, 2 * m] = np.cos(m * phi), np.sin(m * phi)
        Bi[:, 2 * m - 1], Bi[:, 2 * m] = 2 * np.cos(m * phi) / S, 2 * np.sin(m * phi) / S
    return B, Bi


def _dmaj_perm(Ch, D):
    # index array p with p[d*Ch + c] = c*D + d  (ref (c,d)-flat -> d-major)
    p = np.empty(Ch * D, np.int64)
    for d in range(D):
        for c in range(Ch):
            p[d * Ch + c] = c * D + d
    return p


def prep(inputs):
    """Pure-numpy preprocessing. Returns dict of device arrays + meta."""
    x = np.asarray(inputs['x'], np.float32)                    # [N,7,5]
    ec = np.asarray(inputs['edge_coords'], np.float32)
    theta = np.asarray(inputs['connection'], np.float32)
    ei = np.asarray(inputs['edge_index'], np.int64)
    dst, src = ei[0], ei[1]
    batch = np.asarray(inputs['batch'], np.int64)

    # Pf [E, 10]  (k = r*5 + a), matching reference gem_precomp reshape order
    r, th = ec[:, 0], ec[:, 1]
    t = np.clip(r, 0.0, 1.0)
    radial = np.stack([1.0 - t, t], 1)
    ang = np.stack([np.ones_like(th), np.cos(th), np.sin(th),
                    np.cos(2.0 * th), np.sin(2.0 * th)], 1)
    Pf = (radial[:, :, None] * ang[:, None, :]).reshape(N_EDGES, K)

    deg = np.bincount(dst, minlength=N_NODES)

    # ---- bucket packing per core (LPT: least-loaded of NB bins) ----
    import heapq
    NB = 240                 # fixed: keeps global slot ids < 32768 (int16 gather)
    core_node_lists = []
    for c in range(NC):
        nodes = np.arange(c * NPC, (c + 1) * NPC)
        order = nodes[np.argsort(-deg[nodes], kind='stable')]
        bins = [[0, 0, []] for _ in range(NB)]    # [edges, slots, nodes]
        heap = [(0, i) for i in range(NB)]
        heapq.heapify(heap)
        spill = []
        for n in order:
            d = int(deg[n])
            tmp = []
            placed = False
            while heap:
                load, i = heapq.heappop(heap)
                b = bins[i]
                if load != b[0]:        # stale entry
                    continue
                if b[1] < BUCK_SLOTS and b[0] + d <= BUCK_EDGES:
                    b[0] += d; b[1] += 1; b[2].append(n)
                    if b[1] < BUCK_SLOTS:
                        heapq.heappush(heap, (b[0], i))
                    placed = True
                    break
                tmp.append((load, i))
            for t in tmp:
                heapq.heappush(heap, t)
            assert placed, "bucket packing overflow; raise NB"
        core_node_lists.append([b[2] for b in bins])
    SPC = NB * BUCK_SLOTS               # slots per core
    NCHUNK = NB // CHUNK_BUCKS
    NGRP = NB // GRP
    assert NB % GRP == 0 and SPC % TBLK == 0

    # ---- slot assignment ----
    node_slot = np.full(N_NODES, -1, np.int64)
    slot_node = np.full((NC, SPC), -1, np.int64)
    for c in range(NC):
        for bi, bnodes in enumerate(core_node_lists[c]):
            for j, n in enumerate(bnodes):
                s = bi * BUCK_SLOTS + j
                slot_node[c, s] = n
                node_slot[n] = c * SPC + s
    assert (node_slot[:N_NODES] >= 0).all()

    # ---- per-core edge lists per bucket ----
    # edges sorted by (core, bucket, slot)
    e_of_node = [[] for _ in range(N_NODES)]
    for e in range(N_EDGES):
        e_of_node[dst[e]].append(e)

    idx_t = np.zeros((NC, BUCK_EDGES, NB), np.int32)       # src global slot
    A_t = np.zeros((NC, BUCK_EDGES, NB * BUCK_SLOTS * K), BF16)
    # trig tables per group layout: per bucket, rot cols (4 blocks of Ch)
    trig = {}
    for Ch in (7, 16):
        W4 = 4 * Ch
        trig[Ch] = (np.ones((NC, BUCK_EDGES, NB * W4), BF16),
                    np.zeros((NC, BUCK_EDGES, NB * W4), BF16))
    c1, s1 = np.cos(theta), np.sin(theta)
    c2, s2 = np.cos(2 * theta), np.sin(2 * theta)

    for c in range(NC):
        for bi, bnodes in enumerate(core_node_lists[c]):
            rows = 0
            for j, n in enumerate(bnodes):
                for e in e_of_node[n]:
                    p = rows; rows += 1
                    idx_t[c, p, bi] = node_slot[src[e]]
                    A_t[c, p, bi * 160 + j:bi * 160 + 160:BUCK_SLOTS] = Pf[e]
                    for Ch in (7, 16):
                        TC, TS = trig[Ch]
                        base = bi * 4 * Ch
                        TC[c, p, base:base + Ch] = c1[e]
                        TC[c, p, base + Ch:base + 2 * Ch] = c1[e]
                        TC[c, p, base + 2 * Ch:base + 3 * Ch] = c2[e]
                        TC[c, p, base + 3 * Ch:base + 4 * Ch] = c2[e]
                        TS[c, p, base:base + Ch] = -s1[e]
                        TS[c, p, base + Ch:base + 2 * Ch] = s1[e]
                        TS[c, p, base + 2 * Ch:base + 3 * Ch] = -s2[e]
                        TS[c, p, base + 3 * Ch:base + 4 * Ch] = s2[e]
            assert rows <= BUCK_EDGES

    # ---- x0 in slot order, d-major ----
    pin0 = _dmaj_perm(7, 5)
    x0dm = x.reshape(N_NODES, 35)[:, pin0]                  # [N, 35] d-major
    x0_full = np.zeros((NC * SPC, 35), BF16)
    valid = slot_node.reshape(-1) >= 0
    x0_full[valid] = x0dm[slot_node.reshape(-1)[valid]].astype(BF16)
    x0T = np.zeros((NC, 35, SPC), BF16)
    for c in range(NC):
        x0T[c] = x0_full[c * SPC:(c + 1) * SPC].T

    # ---- weights (d-major, lhsT layouts) ----
    def wstack(Wref, Cin, Din, Cout, Dout):
        Fin, Fout = Cin * Din, Cout * Dout
        pin = _dmaj_perm(Cin, Din)
        pout = _dmaj_perm(Cout, Dout)
        Wd = np.asarray(Wref, np.float32)[:, pout][:, :, pin]   # [K, Fout, Fin] d-major
        Ws = np.zeros((Fin, K * Fout), np.float32)
        for k in range(K):
            Ws[:, k * Fout:(k + 1) * Fout] = Wd[k].T
        return Ws.astype(BF16)

    Wst = [
        wstack(inputs['b1c1_W'], 7, 5, 16, 5),
        wstack(inputs['b1c2_W'], 16, 5, 16, 5),
        wstack(inputs['b2c1_W'], 16, 5, 16, 5),
        wstack(inputs['b2c2_W'], 16, 5, 16, 5),
        wstack(inputs['b3c1_W'], 16, 5, 16, 1),
        wstack(inputs['b3c2_W'], 16, 1, 16, 1),
    ]

    ORD = [0, 1, 1, 2, 2]
    SC1 = np.zeros((35, 80), np.float32)
    b1sc = np.asarray(inputs['b1sc_W'], np.float32)         # [3, 16, 7]
    for d in range(5):
        for ci in range(7):
            for co in range(16):
                SC1[d * 7 + ci, d * 16 + co] = b1sc[ORD[d], co, ci]
    SC1 = SC1.astype(BF16)
    I80 = np.eye(80, dtype=np.float32).astype(BF16)
    SC3 = np.asarray(inputs['b3sc_W'], np.float32).T.astype(BF16)   # [ci, co]

    B2, Bi2 = _fourier(2)
    Mb = np.zeros((80, 112), np.float32)
    Mbi = np.zeros((112, 80), np.float32)
    for d in range(5):
        for j in range(7):
            for cch in range(16):
                Mb[d * 16 + cch, j * 16 + cch] = B2[j, d]
                Mbi[j * 16 + cch, d * 16 + cch] = Bi2[j, d]
    Mb = Mb.astype(BF16); Mbi = Mbi.astype(BF16)

    biases = np.zeros((128, 6), np.float32)
    for i, nm in enumerate(['b1c1_b', 'b1c2_b', 'b2c1_b', 'b2c2_b', 'b3c1_b', 'b3c2_b']):
        b = np.asarray(inputs[nm], np.float32)
        biases[:len(b), i] = b

    lin1s = np.asarray(inputs['lin1_W'], np.float32).T.astype(BF16)      # [16, 256]
    lin2T = np.asarray(inputs['lin2_W'], np.float32).T                   # [256, 40]
    lin2a = lin2T[:128].astype(BF16)
    lin2b = lin2T[128:].astype(BF16)
    lin1b = np.zeros((128, 2), np.float32)
    lin1b[:, 0] = np.asarray(inputs['lin1_b'], np.float32)[:128]
    lin1b[:, 1] = np.asarray(inputs['lin1_b'], np.float32)[128:]

    # pooling select matrices [NC, TBLK, (SPC/TBLK)*G]
    nblk = SPC // TBLK
    gsel = np.zeros((NC, TBLK, nblk * G), BF16)
    for c in range(NC):
        for s in range(SPC):
            n = slot_node[c, s]
            if n >= 0:
                gsel[c, s % TBLK, (s // TBLK) * G + batch[n]] = 1.0
    counts = np.bincount(batch, minlength=G).astype(np.float32)
    invcnt = (1.0 / np.maximum(counts, 1.0)).reshape(G, 1).astype(np.float32)
    bias2 = np.broadcast_to(np.asarray(inputs['lin2_b'], np.float32), (G, 40)).copy()

    ident = np.eye(128, dtype=np.float32).astype(BF16)

    shared = dict(
        x0_full=x0_full, w1=Wst[0], w2=Wst[1], w3=Wst[2], w4=Wst[3],
        w5=Wst[4], w6=Wst[5], sc1=SC1, i80=I80, sc3=SC3, mb=Mb, mbi=Mbi,
        biases=biases, lin1s=lin1s, lin2a=lin2a, lin2b=lin2b, lin1b=lin1b,
        invcnt=invcnt, bias2=bias2, ident=ident,
    )
    percore = dict(
        a_t=A_t, idx_t=idx_t,
        tc7=trig[7][0], ts7=trig[7][1], tc16=trig[16][0], ts16=trig[16][1],
        x0t=x0T, gsel=gsel,
    )
    meta = dict(NB=NB, SPC=SPC, NCHUNK=NCHUNK, NGRP=NGRP, nblk=nblk)
    return shared, percore, meta


# ----------------------------------------------------------------------------
# device program
# ----------------------------------------------------------------------------

def build(meta):
    NB, SPC, NCHUNK, NGRP, nblk = meta['NB'], meta['SPC'], meta['NCHUNK'], meta['NGRP'], meta['nblk']
    dt = mybir.dt
    BF = dt.bfloat16
    F32 = dt.float32
    Alu = mybir.AluOpType
    Act = mybir.ActivationFunctionType
    RG = [list(range(NC))]

    nc = bacc.Bacc("TRN2", target_bir_lowering=False, debug=False, num_devices=NC)

    ein = {}
    def EI(name, shape, dtype=BF):
        ein[name] = nc.dram_tensor(name, shape, dtype, kind="ExternalInput").ap()

    EI('x0_full', [NC * SPC, 35])
    EI('a_t', [128, NB * 160])
    EI('idx_t', [128, NB], dt.int32)
    EI('tc7', [128, NB * 28]); EI('ts7', [128, NB * 28])
    EI('tc16', [128, NB * 64]); EI('ts16', [128, NB * 64])
    EI('x0t', [35, SPC])
    EI('gsel', [TBLK, nblk * G])
    EI('w1', [35, 800]); EI('w2', [80, 800]); EI('w3', [80, 800])
    EI('w4', [80, 800]); EI('w5', [80, 160]); EI('w6', [16, 160])
    EI('sc1', [35, 80]); EI('i80', [80, 80]); EI('sc3', [16, 16])
    EI('mb', [80, 112]); EI('mbi', [112, 80])
    EI('biases', [128, 6], F32)
    EI('lin1s', [16, 256]); EI('lin2a', [128, 40]); EI('lin2b', [128, 40])
    EI('lin1b', [128, 2], F32)
    EI('invcnt', [64, 1], F32); EI('bias2', [64, 40], F32)
    EI('ident', [128, 128])
    out_ext = nc.dram_tensor('out', [G, 40], F32, kind="ExternalOutput").ap()

    with tile.TileContext(nc) as tc:
        with (
            tc.tile_pool(name="sb", bufs=1) as sb,
            tc.tile_pool(name="pp", bufs=1, space="PSUM") as pp,
            tc.tile_pool(name="dp", bufs=1, space="DRAM") as dp,
        ):
            # ---- load constants ----
            csb = {}
            for name, P in [('a_t', 128), ('idx_t', 128), ('x0t', 35),
                            ('gsel', TBLK), ('w1', 35), ('w2', 80), ('w3', 80),
                            ('w4', 80), ('w5', 80), ('w6', 16), ('sc1', 35),
                            ('i80', 80), ('sc3', 16), ('mb', 80), ('mbi', 112),
                            ('biases', 128), ('lin1s', 16), ('lin2a', 128),
                            ('lin2b', 128), ('lin1b', 128), ('invcnt', 64),
                            ('bias2', 64), ('ident', 128)]:
                ap = ein[name]
                t = sb.tile(list(ap.shape), ap.dtype, tag=f"c_{name}", bufs=1,
                            name=f"c_{name}")
                nc.sync.dma_start(out=t[:, :], in_=ap[:, :])
                csb[name] = t

            w_of_conv = ['w1', 'w2', 'w3', 'w4', 'w5', 'w6']
            xT_by_conv = [None] * 6

            def emit_group(g, ci, src_full):
                Fin, Cin, in_order = CONVS[ci][0], CONVS[ci][1], CONVS[ci][2]
                xgb = sb.tile([128, GRP * 80], BF, tag='xgb', bufs=3, name=f'xgb{ci}_{g}')
                for lane in range(GRP):
                    b = g * GRP + lane
                    nc.gpsimd.indirect_dma_start(
                        out=xgb[:, lane * 80:lane * 80 + Fin],
                        out_offset=None,
                        in_=src_full,
                        in_offset=bass.IndirectOffsetOnAxis(
                            ap=csb['idx_t'][:, b:b + 1], axis=0),
                    )
                if in_order == 2:
                    W4 = 4 * Cin
                    tck, tsk = (f'tc{Cin}', f'ts{Cin}')
                    tcg = sb.tile([128, GRP * W4], BF, tag='tcg', bufs=2, name=f'tc{ci}_{g}')
                    tsg = sb.tile([128, GRP * W4], BF, tag='tsg', bufs=2, name=f'ts{ci}_{g}')
                    nc.sync.dma_start(out=tcg[:, :], in_=ein[tck][:, g * GRP * W4:(g + 1) * GRP * W4])
                    nc.sync.dma_start(out=tsg[:, :], in_=ein[tsk][:, g * GRP * W4:(g + 1) * GRP * W4])
                    tmp = sb.tile([128, GRP * W4], BF, tag='tmp', bufs=2, name=f'tmp{ci}_{g}')
                    xg_r = xgb[:, :].rearrange("p (b f) -> p b f", f=80)
                    tmp_r = tmp[:, :].rearrange("p (b q) -> p b q", q=W4)
                    tc_r = tcg[:, :].rearrange("p (b q) -> p b q", q=W4)
                    ts_r = tsg[:, :].rearrange("p (b q) -> p b q", q=W4)
                    # tmp[blk t] = xg[swap-src blk] * TS[blk t]
                    for tb, sblk in [(0, 2), (1, 1), (2, 4), (3, 3)]:
                        nc.vector.tensor_tensor(
                            out=tmp_r[:, :, tb * Cin:(tb + 1) * Cin],
                            in0=xg_r[:, :, sblk * Cin:(sblk + 1) * Cin],
                            in1=ts_r[:, :, tb * Cin:(tb + 1) * Cin],
                            op=Alu.mult)
                    nc.vector.tensor_tensor(
                        out=xg_r[:, :, Cin:5 * Cin], in0=xg_r[:, :, Cin:5 * Cin],
                        in1=tc_r[:, :, :], op=Alu.mult)
                    nc.vector.tensor_tensor(
                        out=xg_r[:, :, Cin:5 * Cin], in0=xg_r[:, :, Cin:5 * Cin],
                        in1=tmp_r[:, :, :], op=Alu.add)
                return xgb

            def do_conv(ci, src_full, sc_name, sc_src):
                Fin, Cin, in_order, Fout, Cout, out_order = CONVS[ci]
                Wsb = csb[w_of_conv[ci]]
                xT = sb.tile([80, SPC], BF, tag='xT', bufs=3, name=f'xT{ci}')
                xT_by_conv[ci] = xT
                produce_rows = ci < 5
                if produce_rows:
                    xrows = sb.tile([TBLK, nblk * Fout], BF, tag='xrows', bufs=2,
                                    name=f'xrows{ci}')
                xgb = None
                for ch in range(NCHUNK):
                    S_sb = sb.tile([128, CHUNK_BUCKS * 160], BF, tag='Schunk',
                                   bufs=2, name=f'S{ci}_{ch}')
                    for pt in range(10):
                        ps = pp.tile([128, 480], F32, tag='scat', bufs=2, name=f'ps{ci}_{ch}_{pt}')
                        for j3 in range(3):
                            b = ch * CHUNK_BUCKS + pt * 3 + j3
                            if b % GRP == 0:
                                xgb = emit_group(b // GRP, ci, src_full)
                            lane = b % GRP
                            nc.tensor.matmul(
                                ps[:Fin, j3 * 160:(j3 + 1) * 160],
                                lhsT=xgb[:, lane * 80:lane * 80 + Fin],
                                rhs=csb['a_t'][:, b * 160:(b + 1) * 160],
                                start=(j3 == 0), stop=(j3 == 2))
                        # psum cols (b,k,s) -> S cols k*480 + (pt*3+b)*16 + s
                        s_view = S_sb[:Fin, :].rearrange(
                            "p (k b s) -> p b k s", k=K, s=BUCK_SLOTS)
                        dst = s_view[:, pt * 3:(pt + 1) * 3, :, :]
                        srcv = ps[:Fin, :480].rearrange(
                            "p (b k s) -> p b k s", b=3, k=K)
                        if pt % 2 == 0:
                            nc.scalar.copy(out=dst, in_=srcv)
                        else:
                            nc.vector.tensor_copy(out=dst, in_=srcv)
                    # node phase
                    outp = pp.tile([128, 480], F32, tag='nodep', bufs=2, name=f'op{ci}_{ch}')
                    have_sc = sc_name is not None
                    for k in range(K):
                        nc.tensor.matmul(
                            outp[:Fout, :480],
                            lhsT=Wsb[:Fin, k * Fout:(k + 1) * Fout],
                            rhs=S_sb[:Fin, k * 480:(k + 1) * 480],
                            start=(k == 0), stop=(k == K - 1 and not have_sc))
                    if have_sc:
                        scm = csb[sc_name]
                        FinSC = scm.shape[0]
                        nc.tensor.matmul(
                            outp[:Fout, :480], lhsT=scm[:, :],
                            rhs=sc_src[:FinSC, ch * 480:(ch + 1) * 480],
                            start=False, stop=True)
                    if ci <= 3:
                        t1 = sb.tile([128, 480], BF, tag='t1', bufs=2, name=f't1_{ci}_{ch}')
                        nc.scalar.activation(out=t1[:Fout, :480], in_=outp[:Fout, :480],
                                             func=Act.Identity,
                                             bias=csb['biases'][:Fout, ci:ci + 1])
                        spp = pp.tile([128, 480], F32, tag='sp', bufs=1, name=f'sp{ci}_{ch}')
                        nc.tensor.matmul(spp[:112, :480], lhsT=csb['mb'][:, :],
                                         rhs=t1[:80, :480], start=True, stop=True)
                        t2 = sb.tile([128, 480], BF, tag='t2', bufs=2, name=f't2_{ci}_{ch}')
                        nc.scalar.activation(out=t2[:112, :480], in_=spp[:112, :480],
                                             func=Act.Relu)
                        xpp = pp.tile([128, 480], F32, tag='xp', bufs=1, name=f'xp{ci}_{ch}')
                        nc.tensor.matmul(xpp[:80, :480], lhsT=csb['mbi'][:, :],
                                         rhs=t2[:112, :480], start=True, stop=True)
                        nc.vector.tensor_copy(out=xT[:80, ch * 480:(ch + 1) * 480],
                                              in_=xpp[:80, :480])
                    else:
                        nc.scalar.activation(out=xT[:16, ch * 480:(ch + 1) * 480],
                                             in_=outp[:16, :480], func=Act.Relu,
                                             bias=csb['biases'][:16, ci:ci + 1])
                    if produce_rows:
                        for j in range(480 // TBLK):
                            tp = pp.tile([128, 128], BF, tag='tp', bufs=2, name=f'tp{ci}_{ch}_{j}')
                            cs = ch * 480 + j * TBLK
                            nc.tensor.transpose(tp[:TBLK, :Fout],
                                                in_=xT[:Fout, cs:cs + TBLK],
                                                identity=csb['ident'][:Fout, :Fout])
                            blk = ch * (480 // TBLK) + j
                            nc.vector.tensor_copy(
                                out=xrows[:TBLK, blk * Fout:(blk + 1) * Fout],
                                in_=tp[:TBLK, :Fout])
                # store + allgather
                if produce_rows:
                    xloc = dp.tile([SPC, Fout], BF, tag=f'xloc{Fout}', bufs=2,
                                   name=f'xloc{ci}')
                    nc.sync.dma_start(
                        out=xloc[:, :].rearrange("(b p) f -> p b f", p=TBLK),
                        in_=xrows[:TBLK, :].rearrange("p (b f) -> p b f", f=Fout))
                    xfull = dp.tile([NC * SPC, Fout], BF, tag=f'xfull{Fout}', bufs=2,
                                    name=f'xfull{ci}', addr_space="Shared")
                    nc.gpsimd.collective_compute(
                        "AllGather", Alu.bypass, replica_groups=RG,
                        ins=[xloc.opt()], outs=[xfull.opt()])
                    return xfull
                return None

            h1f = do_conv(0, ein['x0_full'], None, None)
            x1f = do_conv(1, h1f, 'sc1', csb['x0t'])
            h2f = do_conv(2, x1f, None, None)
            x2f = do_conv(3, h2f, 'i80', xT_by_conv[1])
            h3f = do_conv(4, x2f, None, None)
            do_conv(5, h3f, 'sc3', xT_by_conv[3])
            x3T = xT_by_conv[5]

            # ---- head ----
            pool_ps = pp.tile([64, 40], F32, tag='scat', bufs=2, name='pool_ps')
            for ch in range(NCHUNK):
                fa = pp.tile([128, 480], F32, tag='nodep', bufs=2, name=f'fa{ch}')
                fb = pp.tile([128, 480], F32, tag='nodep', bufs=2, name=f'fb{ch}')
                nc.tensor.matmul(fa[:128, :480], lhsT=csb['lin1s'][:, 0:128],
                                 rhs=x3T[:16, ch * 480:(ch + 1) * 480],
                                 start=True, stop=True)
                nc.tensor.matmul(fb[:128, :480], lhsT=csb['lin1s'][:, 128:256],
                                 rhs=x3T[:16, ch * 480:(ch + 1) * 480],
                                 start=True, stop=True)
                ta = sb.tile([128, 480], BF, tag='t1', bufs=2, name=f'ta{ch}')
                tb = sb.tile([128, 480], BF, tag='t2', bufs=2, name=f'tb{ch}')
                nc.scalar.activation(out=ta[:128, :480], in_=fa[:128, :480],
                                     func=Act.Relu, bias=csb['lin1b'][:, 0:1])
                nc.scalar.activation(out=tb[:128, :480], in_=fb[:128, :480],
                                     func=Act.Relu, bias=csb['lin1b'][:, 1:2])
                lp = pp.tile([128, 480], F32, tag='sp', bufs=1, name=f'lp{ch}')
                nc.tensor.matmul(lp[:40, :480], lhsT=csb['lin2a'][:, :],
                                 rhs=ta[:128, :480], start=True, stop=False)
                nc.tensor.matmul(lp[:40, :480], lhsT=csb['lin2b'][:, :],
                                 rhs=tb[:128, :480], start=False, stop=True)
                lg = sb.tile([128, 480], BF, tag='lg', bufs=2, name=f'lg{ch}')
                nc.scalar.copy(out=lg[:40, :480], in_=lp[:40, :480])
                for j in range(480 // TBLK):
                    tp = pp.tile([128, 128], BF, tag='tp', bufs=2, name=f'htp{ch}_{j}')
                    nc.tensor.transpose(tp[:TBLK, :40],
                                        in_=lg[:40, j * TBLK:(j + 1) * TBLK],
                                        identity=csb['ident'][:40, :40])
                    lgr = sb.tile([128, 40], BF, tag='lgr', bufs=2, name=f'lgr{ch}_{j}')
                    nc.vector.tensor_copy(out=lgr[:TBLK, :40], in_=tp[:TBLK, :40])
                    jg = ch * (480 // TBLK) + j
                    nc.tensor.matmul(pool_ps[:64, :40],
                                     lhsT=csb['gsel'][:, jg * G:(jg + 1) * G],
                                     rhs=lgr[:TBLK, :40],
                                     start=(jg == 0), stop=(jg == nblk - 1),
                                     skip_group_check=True)

            psb = sb.tile([64, 40], F32, tag='psb', bufs=1, name='psb')
            nc.vector.tensor_copy(out=psb[:, :], in_=pool_ps[:64, :40])
            arin = dp.tile([64, 40], F32, tag='arin', bufs=1, name='arin')
            nc.sync.dma_start(out=arin[:, :], in_=psb[:, :])
            arout = dp.tile([64, 40], F32, tag='arout', bufs=1, name='arout',
                            addr_space="Shared")
            nc.gpsimd.collective_compute("AllReduce", Alu.add, replica_groups=RG,
                                         ins=[arin.opt()], outs=[arout.opt()])
            pall = sb.tile([64, 40], F32, tag='pall', bufs=1, name='pall')
            nc.sync.dma_start(out=pall[:, :], in_=arout[:, :])
            pm = sb.tile([64, 40], F32, tag='pm', bufs=1, name='pm')
            nc.vector.tensor_scalar_mul(out=pm[:, :], in0=pall[:, :],
                                        scalar1=csb['invcnt'][:, 0:1])
            nc.vector.tensor_tensor(out=pm[:, :], in0=pm[:, :],
                                    in1=csb['bias2'][:, :], op=Alu.add)
            mx = sb.tile([64, 1], F32, tag='mx', bufs=1, name='mx')
            nc.vector.tensor_reduce(out=mx[:, :], in_=pm[:, :],
                                    axis=mybir.AxisListType.X, op=Alu.max)
            sh = sb.tile([64, 40], F32, tag='sh', bufs=1, name='sh')
            nc.vector.tensor_scalar(out=sh[:, :], in0=pm[:, :],
                                    scalar1=mx[:, 0:1], scalar2=None,
                                    op0=Alu.subtract)
            exv = sb.tile([64, 40], F32, tag='exv', bufs=1, name='exv')
            nc.scalar.activation(out=exv[:, :], in_=sh[:, :], func=Act.Exp)
            sm = sb.tile([64, 1], F32, tag='sm', bufs=1, name='sm')
            nc.vector.tensor_reduce(out=sm[:, :], in_=exv[:, :],
                                    axis=mybir.AxisListType.X, op=Alu.add)
            lns = sb.tile([64, 1], F32, tag='lns', bufs=1, name='lns')
            nc.scalar.activation(out=lns[:, :], in_=sm[:, :], func=Act.Ln)
            ov = sb.tile([64, 40], F32, tag='ov', bufs=1, name='ov')
            nc.vector.tensor_scalar(out=ov[:, :], in0=sh[:, :],
                                    scalar1=lns[:, 0:1], scalar2=None,
                                    op0=Alu.subtract)
            nc.sync.dma_start(out=out_ext[:, :], in_=ov[:, :])

    nc.compile()
    return nc


def make_in_maps(shared, percore):
    in_maps = []
    for c in range(NC):
        m = {k: np.ascontiguousarray(v) for k, v in shared.items()}
        for k, v in percore.items():
            m[k] = np.ascontiguousarray(v[c])
        in_maps.append(m)
    return in_maps


def kernel(**inputs):
    shared, percore, meta = prep(inputs)
    nc = build(meta)
    in_maps = make_in_maps(shared, percore)
    res = run_bass_kernel_spmd(nc, in_maps, core_ids=list(range(NC)))
    return np.asarray(res.results[0]['out'], np.float32)



# revision 2
# speedup vs baseline: 2.1297x; 2.1297x over previous
"""GNN message-passing (GEM-CNN style) Trainium2 kernel.

Strategy (node-sharded, scatter-first), v2:
  - 8 cores, core c owns a contiguous slice of nodes packed into 20-slot
    "buckets" with <=128 incident (incoming) edges each; per bucket a dense
    [128, 200] matrix A with A[edge_row, k*20 + slot_local] = Pf[edge, k], so
    a single PE matmul  xg_rot^T @ A  scatter-accumulates S^T in PSUM.
  - Gathers are batched: ONE indirect DMA per 24-bucket chunk ([128, 24]
    offsets -> [128, 24*Fin] tile) instead of per-bucket calls; SWDGE cost is
    ~1us fixed per instruction, so batching cuts gpsimd time ~25x.
  - Parallel-transport trig is stored compactly ([128, NB*4]: c1,s1,c2,s2 per
    bucket column) and broadcast across channels with stride-0 APs on the
    vector engine; tables stay resident in SBUF (no per-conv HBM reload).
  - Node phase: out^T[o, n] = sum_k Wk^T S_k^T (+ shortcut matmul + bias),
    then reg_relu as two more matmuls with a fused ACT relu, all in [f, n]
    layout; PE transposes convert back to row layout for the next gather.
  - AllGather of node-feature shards between convs; AllReduce of the tiny
    pooled [64, 40] logits at the end; log_softmax on-chip.

All features/weights in bf16 (validated: ~2e-4 rel err), PSUM accum fp32.
"""
import numpy as np
import ml_dtypes

import concourse.bass as bass
import concourse.bacc as bacc
import concourse.mybir as mybir
import concourse.tile as tile
from concourse.bass_utils import run_bass_kernel_spmd

BF16 = ml_dtypes.bfloat16

N_NODES, N_EDGES, G = 30000, 180000, 64
NC = 8
NPC = N_NODES // NC            # nodes per core
BUCK_SLOTS = 20                # node slots per bucket
BUCK_EDGES = 128               # max edges per bucket
K = 10                         # precomp channels (2 rings x 5 ang)
BCOLS = BUCK_SLOTS * K         # A columns per bucket (200)
CHUNK_BUCKS = 24               # buckets per chunk (480 slots)
PSB = 2                        # buckets per scatter-psum tile (400 cols)
TBLK = 120                     # transpose block (slots)

# conv table: (Fin, Cin, in_order, Fout, Cout, out_order)
CONVS = [
    (35, 7, 2, 80, 16, 2),   # b1c1
    (80, 16, 2, 80, 16, 2),  # b1c2
    (80, 16, 2, 80, 16, 2),  # b2c1
    (80, 16, 2, 80, 16, 2),  # b2c2
    (80, 16, 2, 16, 16, 0),  # b3c1
    (16, 16, 0, 16, 16, 0),  # b3c2
]


# ----------------------------------------------------------------------------
# host-side helpers
# ----------------------------------------------------------------------------

def _fourier(order, S=7):
    phi = np.arange(S) * (2.0 * np.pi / S)
    D = 2 * order + 1
    B = np.ones((S, D), np.float32)
    Bi = np.full((S, D), 1.0 / S, np.float32)
    for m in range(1, order + 1):
        B[:, 2 * m - 1], B[:, 2 * m] = np.cos(m * phi), np.sin(m * phi)
        Bi[:, 2 * m - 1], Bi[:, 2 * m] = 2 * np.cos(m * phi) / S, 2 * np.sin(m * phi) / S
    return B, Bi


def _dmaj_perm(Ch, D):
    # index array p with p[d*Ch + c] = c*D + d  (ref (c,d)-flat -> d-major)
    p = np.empty(Ch * D, np.int64)
    for d in range(D):
        for c in range(Ch):
            p[d * Ch + c] = c * D + d
    return p


def _pack(deg, NB):
    """LPT pack each core's nodes into NB buckets (<=BUCK_SLOTS nodes,
    <=BUCK_EDGES edges). Returns per-core bucket node lists or None."""
    import heapq
    core_node_lists = []
    for c in range(NC):
        nodes = np.arange(c * NPC, (c + 1) * NPC)
        order = nodes[np.argsort(-deg[nodes], kind='stable')]
        bins = [[0, 0, []] for _ in range(NB)]    # [edges, slots, nodes]
        heap = [(0, i) for i in range(NB)]
        heapq.heapify(heap)
        for n in order:
            d = int(deg[n])
            tmp = []
            placed = False
            while heap:
                load, i = heapq.heappop(heap)
                b = bins[i]
                if load != b[0]:        # stale entry
                    continue
                if b[1] < BUCK_SLOTS and b[0] + d <= BUCK_EDGES:
                    b[0] += d; b[1] += 1; b[2].append(n)
                    if b[1] < BUCK_SLOTS:
                        heapq.heappush(heap, (b[0], i))
                    placed = True
                    break
                tmp.append((load, i))
            for t in tmp:
                heapq.heappush(heap, t)
            if not placed:
                return None
        core_node_lists.append([b[2] for b in bins])
    return core_node_lists


def prep(inputs):
    """Pure-numpy preprocessing. Returns dict of device arrays + meta."""
    x = np.asarray(inputs['x'], np.float32)                    # [N,7,5]
    ec = np.asarray(inputs['edge_coords'], np.float32)
    theta = np.asarray(inputs['connection'], np.float32)
    ei = np.asarray(inputs['edge_index'], np.int64)
    dst, src = ei[0], ei[1]
    batch = np.asarray(inputs['batch'], np.int64)

    # Pf [E, 10]  (k = r*5 + a), matching reference gem_precomp reshape order
    r, th = ec[:, 0], ec[:, 1]
    t = np.clip(r, 0.0, 1.0)
    radial = np.stack([1.0 - t, t], 1)
    ang = np.stack([np.ones_like(th), np.cos(th), np.sin(th),
                    np.cos(2.0 * th), np.sin(2.0 * th)], 1)
    Pf = (radial[:, :, None] * ang[:, None, :]).reshape(N_EDGES, K)

    deg = np.bincount(dst, minlength=N_NODES)

    # ---- bucket packing per core (LPT; NB must be mult of CHUNK_BUCKS and
    # give SPC % TBLK == 0) ----
    core_node_lists = None
    for NB in (192, 216, 240):
        core_node_lists = _pack(deg, NB)
        if core_node_lists is not None:
            break
    assert core_node_lists is not None, "bucket packing overflow"
    SPC = NB * BUCK_SLOTS               # slots per core
    NCHUNK = NB // CHUNK_BUCKS
    assert NB % CHUNK_BUCKS == 0 and SPC % TBLK == 0

    # ---- slot assignment ----
    node_slot = np.full(N_NODES, -1, np.int64)
    slot_node = np.full((NC, SPC), -1, np.int64)
    for c in range(NC):
        for bi, bnodes in enumerate(core_node_lists[c]):
            for j, n in enumerate(bnodes):
                s = bi * BUCK_SLOTS + j
                slot_node[c, s] = n
                node_slot[n] = c * SPC + s
    assert (node_slot[:N_NODES] >= 0).all()

    # ---- per-core edge lists per bucket ----
    e_of_node = [[] for _ in range(N_NODES)]
    for e in range(N_EDGES):
        e_of_node[dst[e]].append(e)

    idx_t = np.zeros((NC, BUCK_EDGES, NB), np.int32)       # src global slot
    A_t = np.zeros((NC, BUCK_EDGES, NB * BCOLS), BF16)
    # compact trig per bucket column: q = (c1, s1-, c2, s2-) layout below
    tcc = np.ones((NC, BUCK_EDGES, NB * 4), BF16)
    tsc = np.zeros((NC, BUCK_EDGES, NB * 4), BF16)
    c1, s1 = np.cos(theta), np.sin(theta)
    c2, s2 = np.cos(2 * theta), np.sin(2 * theta)

    for c in range(NC):
        for bi, bnodes in enumerate(core_node_lists[c]):
            rows = 0
            for j, n in enumerate(bnodes):
                for e in e_of_node[n]:
                    p = rows; rows += 1
                    idx_t[c, p, bi] = node_slot[src[e]]
                    A_t[c, p, bi * BCOLS + j:bi * BCOLS + BCOLS:BUCK_SLOTS] = Pf[e]
                    base = bi * 4
                    tcc[c, p, base + 0] = c1[e]
                    tcc[c, p, base + 1] = c1[e]
                    tcc[c, p, base + 2] = c2[e]
                    tcc[c, p, base + 3] = c2[e]
                    tsc[c, p, base + 0] = -s1[e]
                    tsc[c, p, base + 1] = s1[e]
                    tsc[c, p, base + 2] = -s2[e]
                    tsc[c, p, base + 3] = s2[e]
            assert rows <= BUCK_EDGES

    # ---- x0 in slot order, d-major ----
    pin0 = _dmaj_perm(7, 5)
    x0dm = x.reshape(N_NODES, 35)[:, pin0]                  # [N, 35] d-major
    x0_full = np.zeros((NC * SPC, 35), BF16)
    valid = slot_node.reshape(-1) >= 0
    x0_full[valid] = x0dm[slot_node.reshape(-1)[valid]].astype(BF16)
    x0T = np.zeros((NC, 35, SPC), BF16)
    for c in range(NC):
        x0T[c] = x0_full[c * SPC:(c + 1) * SPC].T

    # ---- weights (d-major, lhsT layouts) ----
    def wstack(Wref, Cin, Din, Cout, Dout):
        Fin, Fout = Cin * Din, Cout * Dout
        pin = _dmaj_perm(Cin, Din)
        pout = _dmaj_perm(Cout, Dout)
        Wd = np.asarray(Wref, np.float32)[:, pout][:, :, pin]   # [K, Fout, Fin] d-major
        Ws = np.zeros((Fin, K * Fout), np.float32)
        for k in range(K):
            Ws[:, k * Fout:(k + 1) * Fout] = Wd[k].T
        return Ws.astype(BF16)

    Wst = [
        wstack(inputs['b1c1_W'], 7, 5, 16, 5),
        wstack(inputs['b1c2_W'], 16, 5, 16, 5),
        wstack(inputs['b2c1_W'], 16, 5, 16, 5),
        wstack(inputs['b2c2_W'], 16, 5, 16, 5),
        wstack(inputs['b3c1_W'], 16, 5, 16, 1),
        wstack(inputs['b3c2_W'], 16, 1, 16, 1),
    ]

    ORD = [0, 1, 1, 2, 2]
    SC1 = np.zeros((35, 80), np.float32)
    b1sc = np.asarray(inputs['b1sc_W'], np.float32)         # [3, 16, 7]
    for d in range(5):
        for ci in range(7):
            for co in range(16):
                SC1[d * 7 + ci, d * 16 + co] = b1sc[ORD[d], co, ci]
    SC1 = SC1.astype(BF16)
    I80 = np.eye(80, dtype=np.float32).astype(BF16)
    SC3 = np.asarray(inputs['b3sc_W'], np.float32).T.astype(BF16)   # [ci, co]

    B2, Bi2 = _fourier(2)
    Mb = np.zeros((80, 112), np.float32)
    Mbi = np.zeros((112, 80), np.float32)
    for d in range(5):
        for j in range(7):
            for cch in range(16):
                Mb[d * 16 + cch, j * 16 + cch] = B2[j, d]
                Mbi[j * 16 + cch, d * 16 + cch] = Bi2[j, d]
    Mb = Mb.astype(BF16); Mbi = Mbi.astype(BF16)

    biases = np.zeros((128, 6), np.float32)
    for i, nm in enumerate(['b1c1_b', 'b1c2_b', 'b2c1_b', 'b2c2_b', 'b3c1_b', 'b3c2_b']):
        b = np.asarray(inputs[nm], np.float32)
        biases[:len(b), i] = b

    lin1s = np.asarray(inputs['lin1_W'], np.float32).T.astype(BF16)      # [16, 256]
    lin2T = np.asarray(inputs['lin2_W'], np.float32).T                   # [256, 40]
    lin2a = lin2T[:128].astype(BF16)
    lin2b = lin2T[128:].astype(BF16)
    lin1b = np.zeros((128, 2), np.float32)
    lin1b[:, 0] = np.asarray(inputs['lin1_b'], np.float32)[:128]
    lin1b[:, 1] = np.asarray(inputs['lin1_b'], np.float32)[128:]

    # pooling select matrices [NC, TBLK, (SPC/TBLK)*G]
    nblk = SPC // TBLK
    gsel = np.zeros((NC, TBLK, nblk * G), BF16)
    for c in range(NC):
        for s in range(SPC):
            n = slot_node[c, s]
            if n >= 0:
                gsel[c, s % TBLK, (s // TBLK) * G + batch[n]] = 1.0
    counts = np.bincount(batch, minlength=G).astype(np.float32)
    invcnt = (1.0 / np.maximum(counts, 1.0)).reshape(G, 1).astype(np.float32)
    bias2 = np.broadcast_to(np.asarray(inputs['lin2_b'], np.float32), (G, 40)).copy()

    ident = np.eye(128, dtype=np.float32).astype(BF16)

    shared = dict(
        x0_full=x0_full, w1=Wst[0], w2=Wst[1], w3=Wst[2], w4=Wst[3],
        w5=Wst[4], w6=Wst[5], sc1=SC1, i80=I80, sc3=SC3, mb=Mb, mbi=Mbi,
        biases=biases, lin1s=lin1s, lin2a=lin2a, lin2b=lin2b, lin1b=lin1b,
        invcnt=invcnt, bias2=bias2, ident=ident,
    )
    percore = dict(
        a_t=A_t, idx_t=idx_t, tcc=tcc, tsc=tsc,
        x0t=x0T, gsel=gsel,
    )
    meta = dict(NB=NB, SPC=SPC, NCHUNK=NCHUNK, nblk=nblk)
    return shared, percore, meta


# ----------------------------------------------------------------------------
# device program
# ----------------------------------------------------------------------------

def build(meta):
    NB, SPC, NCHUNK, nblk = meta['NB'], meta['SPC'], meta['NCHUNK'], meta['nblk']
    dt = mybir.dt
    BF = dt.bfloat16
    F32 = dt.float32
    Alu = mybir.AluOpType
    Act = mybir.ActivationFunctionType
    RG = [list(range(NC))]
    CB = CHUNK_BUCKS
    NPT = CB // PSB                 # scatter-psum tiles per chunk (12)

    nc = bacc.Bacc("TRN2", target_bir_lowering=False, debug=False, num_devices=NC)

    ein = {}
    def EI(name, shape, dtype=BF):
        ein[name] = nc.dram_tensor(name, shape, dtype, kind="ExternalInput").ap()

    EI('x0_full', [NC * SPC, 35])
    EI('a_t', [128, NB * BCOLS])
    EI('idx_t', [128, NB], dt.int32)
    EI('tcc', [128, NB * 4]); EI('tsc', [128, NB * 4])
    EI('x0t', [35, SPC])
    EI('gsel', [TBLK, nblk * G])
    EI('w1', [35, 800]); EI('w2', [80, 800]); EI('w3', [80, 800])
    EI('w4', [80, 800]); EI('w5', [80, 160]); EI('w6', [16, 160])
    EI('sc1', [35, 80]); EI('i80', [80, 80]); EI('sc3', [16, 16])
    EI('mb', [80, 112]); EI('mbi', [112, 80])
    EI('biases', [128, 6], F32)
    EI('lin1s', [16, 256]); EI('lin2a', [128, 40]); EI('lin2b', [128, 40])
    EI('lin1b', [128, 2], F32)
    EI('invcnt', [64, 1], F32); EI('bias2', [64, 40], F32)
    EI('ident', [128, 128])
    out_ext = nc.dram_tensor('out', [G, 40], F32, kind="ExternalOutput").ap()

    with tile.TileContext(nc) as tc:
        with (
            tc.tile_pool(name="sb", bufs=1) as sb,
            tc.tile_pool(name="pp", bufs=1, space="PSUM") as pp,
            tc.tile_pool(name="dp", bufs=1, space="DRAM") as dp,
        ):
            # ---- load constants ----
            csb = {}
            for name, P in [('a_t', 128), ('idx_t', 128), ('tcc', 128),
                            ('tsc', 128), ('x0t', 35),
                            ('gsel', TBLK), ('w1', 35), ('w2', 80), ('w3', 80),
                            ('w4', 80), ('w5', 80), ('w6', 16), ('sc1', 35),
                            ('i80', 80), ('sc3', 16), ('mb', 80), ('mbi', 112),
                            ('biases', 128), ('lin1s', 16), ('lin2a', 128),
                            ('lin2b', 128), ('lin1b', 128), ('invcnt', 64),
                            ('bias2', 64), ('ident', 128)]:
                ap = ein[name]
                t = sb.tile(list(ap.shape), ap.dtype, tag=f"c_{name}", bufs=1,
                            name=f"c_{name}")
                nc.sync.dma_start(out=t[:, :], in_=ap[:, :])
                csb[name] = t

            w_of_conv = ['w1', 'w2', 'w3', 'w4', 'w5', 'w6']
            xT_by_conv = [None] * 6

            def do_conv(ci, src_full, sc_name, sc_src):
                Fin, Cin, in_order, Fout, Cout, out_order = CONVS[ci]
                Wsb = csb[w_of_conv[ci]]
                xT = sb.tile([80, SPC], BF, tag='xT', bufs=3, name=f'xT{ci}')
                xT_by_conv[ci] = xT
                produce_rows = ci < 5
                if produce_rows:
                    xrows = sb.tile([TBLK, nblk * Fout], BF, tag='xrows', bufs=2,
                                    name=f'xrows{ci}')
                for ch in range(NCHUNK):
                    # ---- batched gather: one indirect DMA per chunk ----
                    xg = sb.tile([128, CB * Fin], BF, tag=f'xg{Fin}', bufs=2,
                                 name=f'xg{ci}_{ch}')
                    nc.gpsimd.indirect_dma_start(
                        out=xg[:, :],
                        out_offset=None,
                        in_=src_full,
                        in_offset=bass.IndirectOffsetOnAxis(
                            ap=csb['idx_t'][:, ch * CB:(ch + 1) * CB], axis=0),
                    )
                    # ---- parallel transport (broadcast compact trig) ----
                    if in_order == 2:
                        xq = xg[:, :].rearrange("p (b d c) -> p b d c", d=5, c=Cin)
                        tmp = sb.tile([128, CB * 4 * Cin], BF, tag=f'tmp{Cin}',
                                      bufs=2, name=f'tmp{ci}_{ch}')
                        tmp_r = tmp[:, :].rearrange("p (b q c) -> p b q c",
                                                    q=4, c=Cin)
                        tq = csb['tcc'][:, ch * CB * 4:(ch + 1) * CB * 4] \
                            .rearrange("p (b q) -> p b q", q=4) \
                            .unsqueeze(3).broadcast_to((128, CB, 4, Cin))
                        sq = csb['tsc'][:, ch * CB * 4:(ch + 1) * CB * 4] \
                            .rearrange("p (b q) -> p b q", q=4) \
                            .unsqueeze(3).broadcast_to((128, CB, 4, Cin))
                        # tmp[q] = xg[swap-src d] * tsc[q]
                        for tb, sd in [(0, 2), (1, 1), (2, 4), (3, 3)]:
                            nc.vector.tensor_tensor(
                                out=tmp_r[:, :, tb, :],
                                in0=xq[:, :, sd, :],
                                in1=sq[:, :, tb, :],
                                op=Alu.mult)
                        nc.vector.tensor_tensor(
                            out=xq[:, :, 1:5, :], in0=xq[:, :, 1:5, :],
                            in1=tq[:, :, :, :], op=Alu.mult)
                        nc.vector.tensor_tensor(
                            out=xq[:, :, 1:5, :], in0=xq[:, :, 1:5, :],
                            in1=tmp_r[:, :, :, :], op=Alu.add)
                    # ---- scatter matmuls into PSUM, copy to S ----
                    S_sb = sb.tile([128, CB * BCOLS], BF, tag='Schunk',
                                   bufs=2, name=f'S{ci}_{ch}')
                    s_view = S_sb[:Fin, :].rearrange(
                        "p (k b s) -> p b k s", k=K, s=BUCK_SLOTS)
                    for pt in range(NPT):
                        ps = pp.tile([128, PSB * BCOLS], F32, tag='scat', bufs=2,
                                     name=f'ps{ci}_{ch}_{pt}')
                        for j2 in range(PSB):
                            bl = pt * PSB + j2
                            b = ch * CB + bl
                            nc.tensor.matmul(
                                ps[:Fin, j2 * BCOLS:(j2 + 1) * BCOLS],
                                lhsT=xg[:, bl * Fin:bl * Fin + Fin],
                                rhs=csb['a_t'][:, b * BCOLS:(b + 1) * BCOLS],
                                start=(j2 == 0), stop=(j2 == PSB - 1))
                        # psum cols (b,k,s) -> S cols k*(CB*S) + b*S + s
                        dstv = s_view[:, pt * PSB:(pt + 1) * PSB, :, :]
                        srcv = ps[:Fin, :].rearrange(
                            "p (b k s) -> p b k s", b=PSB, k=K)
                        if pt % 2 == 0:
                            nc.scalar.copy(out=dstv, in_=srcv)
                        else:
                            nc.vector.tensor_copy(out=dstv, in_=srcv)
                    # ---- node phase ----
                    outp = pp.tile([128, 480], F32, tag='nodep', bufs=2,
                                   name=f'op{ci}_{ch}')
                    have_sc = sc_name is not None
                    for k in range(K):
                        nc.tensor.matmul(
                            outp[:Fout, :480],
                            lhsT=Wsb[:Fin, k * Fout:(k + 1) * Fout],
                            rhs=S_sb[:Fin, k * 480:(k + 1) * 480],
                            start=(k == 0), stop=(k == K - 1 and not have_sc))
                    if have_sc:
                        scm = csb[sc_name]
                        FinSC = scm.shape[0]
                        nc.tensor.matmul(
                            outp[:Fout, :480], lhsT=scm[:, :],
                            rhs=sc_src[:FinSC, ch * 480:(ch + 1) * 480],
                            start=False, stop=True)
                    if ci <= 3:
                        t1 = sb.tile([128, 480], BF, tag='t1', bufs=2, name=f't1_{ci}_{ch}')
                        nc.scalar.activation(out=t1[:Fout, :480], in_=outp[:Fout, :480],
                                             func=Act.Identity,
                                             bias=csb['biases'][:Fout, ci:ci + 1])
                        spp = pp.tile([128, 480], F32, tag='sp', bufs=1, name=f'sp{ci}_{ch}')
                        nc.tensor.matmul(spp[:112, :480], lhsT=csb['mb'][:, :],
                                         rhs=t1[:80, :480], start=True, stop=True)
                        t2 = sb.tile([128, 480], BF, tag='t2', bufs=2, name=f't2_{ci}_{ch}')
                        nc.scalar.activation(out=t2[:112, :480], in_=spp[:112, :480],
                                             func=Act.Relu)
                        xpp = pp.tile([128, 480], F32, tag='xp', bufs=1, name=f'xp{ci}_{ch}')
                        nc.tensor.matmul(xpp[:80, :480], lhsT=csb['mbi'][:, :],
                                         rhs=t2[:112, :480], start=True, stop=True)
                        nc.vector.tensor_copy(out=xT[:80, ch * 480:(ch + 1) * 480],
                                              in_=xpp[:80, :480])
                    else:
                        nc.scalar.activation(out=xT[:16, ch * 480:(ch + 1) * 480],
                                             in_=outp[:16, :480], func=Act.Relu,
                                             bias=csb['biases'][:16, ci:ci + 1])
                    if produce_rows:
                        for j in range(480 // TBLK):
                            tp = pp.tile([128, 128], BF, tag='tp', bufs=2, name=f'tp{ci}_{ch}_{j}')
                            cs = ch * 480 + j * TBLK
                            nc.tensor.transpose(tp[:TBLK, :Fout],
                                                in_=xT[:Fout, cs:cs + TBLK],
                                                identity=csb['ident'][:Fout, :Fout])
                            blk = ch * (480 // TBLK) + j
                            nc.vector.tensor_copy(
                                out=xrows[:TBLK, blk * Fout:(blk + 1) * Fout],
                                in_=tp[:TBLK, :Fout])
                # store + allgather
                if produce_rows:
                    xloc = dp.tile([SPC, Fout], BF, tag=f'xloc{Fout}', bufs=2,
                                   name=f'xloc{ci}')
                    nc.sync.dma_start(
                        out=xloc[:, :].rearrange("(b p) f -> p b f", p=TBLK),
                        in_=xrows[:TBLK, :].rearrange("p (b f) -> p b f", f=Fout))
                    xfull = dp.tile([NC * SPC, Fout], BF, tag=f'xfull{Fout}', bufs=2,
                                    name=f'xfull{ci}', addr_space="Shared")
                    nc.gpsimd.collective_compute(
                        "AllGather", Alu.bypass, replica_groups=RG,
                        ins=[xloc.opt()], outs=[xfull.opt()])
                    return xfull
                return None

            h1f = do_conv(0, ein['x0_full'], None, None)
            x1f = do_conv(1, h1f, 'sc1', csb['x0t'])
            h2f = do_conv(2, x1f, None, None)
            x2f = do_conv(3, h2f, 'i80', xT_by_conv[1])
            h3f = do_conv(4, x2f, None, None)
            do_conv(5, h3f, 'sc3', xT_by_conv[3])
            x3T = xT_by_conv[5]

            # ---- head ----
            pool_ps = pp.tile([64, 40], F32, tag='scat', bufs=2, name='pool_ps')
            for ch in range(NCHUNK):
                fa = pp.tile([128, 480], F32, tag='nodep', bufs=2, name=f'fa{ch}')
                fb = pp.tile([128, 480], F32, tag='nodep', bufs=2, name=f'fb{ch}')
                nc.tensor.matmul(fa[:128, :480], lhsT=csb['lin1s'][:, 0:128],
                                 rhs=x3T[:16, ch * 480:(ch + 1) * 480],
                                 start=True, stop=True)
                nc.tensor.matmul(fb[:128, :480], lhsT=csb['lin1s'][:, 128:256],
                                 rhs=x3T[:16, ch * 480:(ch + 1) * 480],
                                 start=True, stop=True)
                ta = sb.tile([128, 480], BF, tag='t1', bufs=2, name=f'ta{ch}')
                tb = sb.tile([128, 480], BF, tag='t2', bufs=2, name=f'tb{ch}')
                nc.scalar.activation(out=ta[:128, :480], in_=fa[:128, :480],
                                     func=Act.Relu, bias=csb['lin1b'][:, 0:1])
                nc.scalar.activation(out=tb[:128, :480], in_=fb[:128, :480],
                                     func=Act.Relu, bias=csb['lin1b'][:, 1:2])
                lp = pp.tile([128, 480], F32, tag='sp', bufs=1, name=f'lp{ch}')
                nc.tensor.matmul(lp[:40, :480], lhsT=csb['lin2a'][:, :],
                                 rhs=ta[:128, :480], start=True, stop=False)
                nc.tensor.matmul(lp[:40, :480], lhsT=csb['lin2b'][:, :],
                                 rhs=tb[:128, :480], start=False, stop=True)
                lg = sb.tile([128, 480], BF, tag='lg', bufs=2, name=f'lg{ch}')
                nc.scalar.copy(out=lg[:40, :480], in_=lp[:40, :480])
                for j in range(480 // TBLK):
                    tp = pp.tile([128, 128], BF, tag='tp', bufs=2, name=f'htp{ch}_{j}')
                    nc.tensor.transpose(tp[:TBLK, :40],
                                        in_=lg[:40, j * TBLK:(j + 1) * TBLK],
                                        identity=csb['ident'][:40, :40])
                    lgr = sb.tile([128, 40], BF, tag='lgr', bufs=2, name=f'lgr{ch}_{j}')
                    nc.vector.tensor_copy(out=lgr[:TBLK, :40], in_=tp[:TBLK, :40])
                    jg = ch * (480 // TBLK) + j
                    nc.tensor.matmul(pool_ps[:64, :40],
                                     lhsT=csb['gsel'][:, jg * G:(jg + 1) * G],
                                     rhs=lgr[:TBLK, :40],
                                     start=(jg == 0), stop=(jg == nblk - 1),
                                     skip_group_check=True)

            psb = sb.tile([64, 40], F32, tag='psb', bufs=1, name='psb')
            nc.vector.tensor_copy(out=psb[:, :], in_=pool_ps[:64, :40])
            arin = dp.tile([64, 40], F32, tag='arin', bufs=1, name='arin')
            nc.sync.dma_start(out=arin[:, :], in_=psb[:, :])
            arout = dp.tile([64, 40], F32, tag='arout', bufs=1, name='arout',
                            addr_space="Shared")
            nc.gpsimd.collective_compute("AllReduce", Alu.add, replica_groups=RG,
                                         ins=[arin.opt()], outs=[arout.opt()])
            pall = sb.tile([64, 40], F32, tag='pall', bufs=1, name='pall')
            nc.sync.dma_start(out=pall[:, :], in_=arout[:, :])
            pm = sb.tile([64, 40], F32, tag='pm', bufs=1, name='pm')
            nc.vector.tensor_scalar_mul(out=pm[:, :], in0=pall[:, :],
                                        scalar1=csb['invcnt'][:, 0:1])
            nc.vector.tensor_tensor(out=pm[:, :], in0=pm[:, :],
                                    in1=csb['bias2'][:, :], op=Alu.add)
            mx = sb.tile([64, 1], F32, tag='mx', bufs=1, name='mx')
            nc.vector.tensor_reduce(out=mx[:, :], in_=pm[:, :],
                                    axis=mybir.AxisListType.X, op=Alu.max)
            sh = sb.tile([64, 40], F32, tag='sh', bufs=1, name='sh')
            nc.vector.tensor_scalar(out=sh[:, :], in0=pm[:, :],
                                    scalar1=mx[:, 0:1], scalar2=None,
                                    op0=Alu.subtract)
            exv = sb.tile([64, 40], F32, tag='exv', bufs=1, name='exv')
            nc.scalar.activation(out=exv[:, :], in_=sh[:, :], func=Act.Exp)
            sm = sb.tile([64, 1], F32, tag='sm', bufs=1, name='sm')
            nc.vector.tensor_reduce(out=sm[:, :], in_=exv[:, :],
                                    axis=mybir.AxisListType.X, op=Alu.add)
            lns = sb.tile([64, 1], F32, tag='lns', bufs=1, name='lns')
            nc.scalar.activation(out=lns[:, :], in_=sm[:, :], func=Act.Ln)
            ov = sb.tile([64, 40], F32, tag='ov', bufs=1, name='ov')
            nc.vector.tensor_scalar(out=ov[:, :], in0=sh[:, :],
                                    scalar1=lns[:, 0:1], scalar2=None,
                                    op0=Alu.subtract)
            nc.sync.dma_start(out=out_ext[:, :], in_=ov[:, :])

    nc.compile()
    return nc


def make_in_maps(shared, percore):
    in_maps = []
    for c in range(NC):
        m = {k: np.ascontiguousarray(v) for k, v in shared.items()}
        for k, v in percore.items():
            m[k] = np.ascontiguousarray(v[c])
        in_maps.append(m)
    return in_maps


def kernel(**inputs):
    shared, percore, meta = prep(inputs)
    nc = build(meta)
    in_maps = make_in_maps(shared, percore)
    res = run_bass_kernel_spmd(nc, in_maps, core_ids=list(range(NC)))
    return np.asarray(res.results[0]['out'], np.float32)
